# revision 4
# baseline (speedup 1.0000x reference)
"""Trainium2 Bass kernel for nn_EntropyModel (MoE routing over K=4 class towers).

Strategy: every op in the tower is a per-pixel 1x1 conv (matmul over channels),
and the final one-hot masked sum selects exactly one class tower per pixel.
So route on the host: sort pixels by seg class, give each of the 8 cores half
of one class's pixels (expert-parallel, 2 cores per class), run that class's
tower densely on its gathered pixels, and scatter the results back.

The 5-matmul tower is algebraically collapsed to 4 matmuls per pixel by
folding the linear layers around the two LeakyReLUs (host precomputes the
merged 128x128 weights):
    a2 = lrelu(V x + c)          V  = Wr1 W1,      c   = Wr1 b1 + br1
    h3 = lrelu(T x + U a2 + b3') T  = W3 W1,       U   = W3 Wr2,
                                 b3' = W3 (b1 + br2) + b3
    y  = W4 h3 + b4
Matmuls run in float32r (reduced-precision fp32 PE mode, ~1e-4 rel err per
matmul, 4x faster than full fp32).
"""
import numpy as np

import concourse.mybir as mybir
import concourse.tile as tile
from concourse import bacc
from concourse.bass_utils import run_bass_kernel_spmd

B, C, H, W = 2, 128, 192, 192
K = 4
O = 60
NTOT = B * H * W
NCORES = 8
MACRO = 1024  # free-dim per macro tile (2 PSUM banks)
MMF = 512     # free-dim per matmul (1 PSUM bank, fp32)

F32 = mybir.dt.float32
F32R = mybir.dt.float32r

LAST_RESULTS = None  # test harness reads exec_time_ns off this

_nc_cache = {}


def _build(cap):
    nc = bacc.Bacc(None, target_bir_lowering=False)
    x = nc.dram_tensor("x", [C, cap], F32R, kind="ExternalInput")
    # packed weights: [vt | tt | ut | w4t] along free dim
    wp = nc.dram_tensor("wp", [C, 3 * C + O], F32R, kind="ExternalInput")
    # packed biases: [c | b3' | b4(rows 0..59)]
    bp = nc.dram_tensor("bp", [C, 3], F32, kind="ExternalInput")
    y = nc.dram_tensor("y", [O, cap], F32, kind="ExternalOutput")

    spans = []
    s = 0
    while s < cap:
        w = min(MACRO, cap - s)
        spans.append((s, w))
        s += w

    Lrelu = mybir.ActivationFunctionType.Lrelu

    with tile.TileContext(nc) as tc:
        with tc.tile_pool(name="const", bufs=1) as cw, \
             tc.tile_pool(name="xin", bufs=3) as xp, \
             tc.tile_pool(name="mid", bufs=2) as mp, \
             tc.tile_pool(name="yout", bufs=2) as yp, \
             tc.tile_pool(name="ps", bufs=4, space="PSUM") as ps:
            # first x macro goes out before the weights so PE can start early
            xts = []
            for mi, (s, w) in enumerate(spans[:1]):
                xt = xp.tile([C, MACRO], F32R, tag="x", name="xt")[:, :w]
                nc.sync.dma_start(xt[:], x[:, s:s + w])
                xts.append(xt)

            wpt = cw.tile([C, 3 * C + O], F32R)
            nc.sync.dma_start(wpt[:], wp[:])
            bpt = cw.tile([C, 3], F32)
            nc.sync.dma_start(bpt[:], bp[:])
            vtt = wpt[:, 0:C]
            ttt = wpt[:, C:2 * C]
            utt = wpt[:, 2 * C:3 * C]
            w4tt = wpt[:, 3 * C:3 * C + O]
            cbt = bpt[:, 0:1]
            b3t = bpt[:, 1:2]
            b4t = bpt[:O, 2:3]

            for mi, (s, w) in enumerate(spans):
                if mi < len(xts):
                    xt = xts[mi]
                else:
                    xt = xp.tile([C, MACRO], F32R, tag="x", name="xt")[:, :w]
                    nc.sync.dma_start(xt[:], x[:, s:s + w])

                pa = ps.tile([C, MACRO], F32, tag="mm", name="pa")[:, :w]
                for j in range(0, w, MMF):
                    nc.tensor.matmul(pa[:, j:j + MMF], vtt, xt[:, j:j + MMF],
                                     start=True, stop=True)
                a2 = mp.tile([C, MACRO], F32R, tag="a2", name="a2")[:, :w]
                nc.scalar.activation(a2[:], pa[:], Lrelu,
                                     bias=cbt, scale=1.0, alpha=0.01)

                ph = ps.tile([C, MACRO], F32, tag="mm", name="ph")[:, :w]
                for j in range(0, w, MMF):
                    nc.tensor.matmul(ph[:, j:j + MMF], ttt, xt[:, j:j + MMF],
                                     start=True, stop=False)
                for j in range(0, w, MMF):
                    nc.tensor.matmul(ph[:, j:j + MMF], utt, a2[:, j:j + MMF],
                                     start=False, stop=True)
                h3 = mp.tile([C, MACRO], F32R, tag="h3", name="h3")[:, :w]
                nc.scalar.activation(h3[:], ph[:], Lrelu,
                                     bias=b3t, scale=1.0, alpha=0.01)

                py = ps.tile([O, MACRO], F32, tag="mm", name="py")[:, :w]
                for j in range(0, w, MMF):
                    nc.tensor.matmul(py[:, j:j + MMF], w4tt, h3[:, j:j + MMF],
                                     start=True, stop=True)
                yt = yp.tile([O, MACRO], F32, tag="y", name="yt")[:, :w]
                nc.vector.tensor_scalar_add(yt[:], py[:], b4t)
                nc.sync.dma_start(y[:, s:s + w], yt[:])
    nc.compile()
    return nc


def kernel(fusion_context, seg, W1, b1, Wr1, br1, Wr2, br2, W3, b3, W4, b4):
    global LAST_RESULTS
    fusion_context = np.asarray(fusion_context, dtype=np.float32)
    seg = np.asarray(seg)

    # [B,C,H,W] -> [C, B*H*W]; column n = (b, h, w) row-major
    xcols = np.ascontiguousarray(
        fusion_context.transpose(1, 0, 2, 3).reshape(C, NTOT))
    segf = seg.reshape(-1).astype(np.int64)

    # Route: per class index list, split into two halves -> 8 core shards
    shards = []  # (class_id, column_indices)
    for k in range(K):
        ix = np.nonzero(segf == k)[0]
        h = (len(ix) + 1) // 2
        shards.append((k, ix[:h]))
        shards.append((k, ix[h:]))
    assert len(shards) == NCORES

    cap = max(len(ix) for _, ix in shards)
    cap = max(MMF, -(-cap // MMF) * MMF)  # round up to matmul tile

    if cap not in _nc_cache:
        _nc_cache[cap] = _build(cap)
    nc = _nc_cache[cap]

    f64 = np.float64
    in_maps = []
    for k, ix in shards:
        xs = np.zeros((C, cap), dtype=np.float32)
        xs[:, :len(ix)] = xcols[:, ix]
        V = W1[k].astype(f64).T @ Wr1[k].astype(f64).T    # (Wr1 W1)^T
        T = W1[k].astype(f64).T @ W3[k].astype(f64).T     # (W3 W1)^T
        U = Wr2[k].astype(f64).T @ W3[k].astype(f64).T    # (W3 Wr2)^T
        c = Wr1[k].astype(f64) @ b1[k].astype(f64) + br1[k].astype(f64)
        b3p = W3[k].astype(f64) @ (b1[k].astype(f64) + br2[k].astype(f64)) \
            + b3[k].astype(f64)
        wp = np.concatenate(
            [V, T, U, W4[k].T.astype(f64)], axis=1).astype(np.float32)
        bp = np.zeros((C, 3), dtype=np.float32)
        bp[:, 0] = c
        bp[:, 1] = b3p
        bp[:O, 2] = b4[k]
        in_maps.append({
            "x": xs,
            "wp": np.ascontiguousarray(wp),
            "bp": bp,
        })

    res = run_bass_kernel_spmd(nc, in_maps, core_ids=list(range(NCORES)))
    LAST_RESULTS = res

    out = np.empty((O, NTOT), dtype=np.float32)
    for (k, ix), r in zip(shards, res.results):
        out[:, ix] = r["y"][:, :len(ix)]
    return np.ascontiguousarray(
        out.reshape(O, B, H * W).transpose(1, 0, 2).reshape(B, O, H, W))


# revision 6
# speedup vs baseline: 1.2349x; 1.2349x over previous
"""Trainium2 Bass kernel for nn_EntropyModel (MoE routing over K=4 class towers).

Strategy: every op in the tower is a per-pixel 1x1 conv (matmul over channels),
and the final one-hot masked sum selects exactly one class tower per pixel.
So route on the host: sort pixels by seg class, give each of the 8 cores half
of one class's pixels (expert-parallel, 2 cores per class), run that class's
tower densely on its gathered pixels, and scatter the results back.

The 5-matmul tower is algebraically collapsed to 4 matmuls per pixel by
folding the linear layers around the two LeakyReLUs (host precomputes the
merged 128x128 weights):
    a2 = lrelu(V x + c)          V  = Wr1 W1,      c   = Wr1 b1 + br1
    h3 = lrelu(T x + U a2 + b3') T  = W3 W1,       U   = W3 Wr2,
                                 b3' = W3 (b1 + br2) + b3
    y  = W4 h3 + b4
Matmuls run in float32r (reduced-precision fp32 PE mode, ~1e-4 rel err per
matmul, 4x faster than full fp32).
"""
import numpy as np

import concourse.mybir as mybir
import concourse.tile as tile
from concourse import bacc
from concourse.bass_utils import run_bass_kernel_spmd

B, C, H, W = 2, 128, 192, 192
K = 4
O = 60
NTOT = B * H * W
NCORES = 8
MACRO = 2048  # free-dim per ACT/PSUM chunk (4 PSUM banks)
MMF = 512     # free-dim per matmul (1 PSUM bank, fp32)

F32 = mybir.dt.float32
F32R = mybir.dt.float32r

LAST_RESULTS = None  # test harness reads exec_time_ns off this

_nc_cache = {}


def _build(cap):
    nc = bacc.Bacc(None, target_bir_lowering=False)
    x = nc.dram_tensor("x", [C, cap], F32R, kind="ExternalInput")
    # packed weights: [vt | tt | ut | w4t] along free dim
    wp = nc.dram_tensor("wp", [C, 3 * C + O], F32R, kind="ExternalInput")
    # packed biases: [c | b3' | b4(rows 0..59)]
    bp = nc.dram_tensor("bp", [C, 3], F32, kind="ExternalInput")
    y = nc.dram_tensor("y", [O, cap], F32, kind="ExternalOutput")

    spans = []
    s = 0
    while s < cap:
        w = min(MACRO, cap - s)
        spans.append((s, w))
        s += w

    Lrelu = mybir.ActivationFunctionType.Lrelu

    # Layer-sweep structure: PE streams one layer's matmuls across all
    # chunks back-to-back (no per-macro ACT round trips); ACT trails one
    # PSUM slot behind. Intermediates live full-size in SBUF.
    with tile.TileContext(nc) as tc:
        with tc.tile_pool(name="const", bufs=1) as cw, \
             tc.tile_pool(name="big", bufs=1) as bigp, \
             tc.tile_pool(name="ps", bufs=2, space="PSUM") as ps:
            xt = bigp.tile([C, cap], F32R)
            a2t = bigp.tile([C, cap], F32R)
            h3t = bigp.tile([C, cap], F32R)
            yt = bigp.tile([O, cap], F32)

            # first x chunk first so PE can start ASAP; then weights; then rest
            nc.sync.dma_start(xt[:, 0:spans[0][1]], x[:, 0:spans[0][1]])
            wpt = cw.tile([C, 3 * C + O], F32R)
            nc.sync.dma_start(wpt[:], wp[:])
            bpt = cw.tile([C, 3], F32)
            nc.sync.dma_start(bpt[:], bp[:])
            for s, w in spans[1:]:
                nc.sync.dma_start(xt[:, s:s + w], x[:, s:s + w])

            vtt = wpt[:, 0:C]
            ttt = wpt[:, C:2 * C]
            utt = wpt[:, 2 * C:3 * C]
            w4tt = wpt[:, 3 * C:3 * C + O]
            cbt = bpt[:, 0:1]
            b3t = bpt[:, 1:2]
            b4t = bpt[:O, 2:3]

            # sweep 1: a2 = lrelu(V x + c)
            for s, w in spans:
                pa = ps.tile([C, MACRO], F32, tag="mm", name="pa")[:, :w]
                for j in range(s, s + w, MMF):
                    nc.tensor.matmul(pa[:, j - s:j - s + MMF], vtt,
                                     xt[:, j:j + MMF], start=True, stop=True)
                nc.scalar.activation(a2t[:, s:s + w], pa[:], Lrelu,
                                     bias=cbt, scale=1.0, alpha=0.01)

            # sweep 2: h3 = lrelu(T x + U a2 + b3')
            for s, w in spans:
                ph = ps.tile([C, MACRO], F32, tag="mm", name="ph")[:, :w]
                for j in range(s, s + w, MMF):
                    nc.tensor.matmul(ph[:, j - s:j - s + MMF], ttt,
                                     xt[:, j:j + MMF], start=True, stop=False)
                for j in range(s, s + w, MMF):
                    nc.tensor.matmul(ph[:, j - s:j - s + MMF], utt,
                                     a2t[:, j:j + MMF], start=False, stop=True)
                nc.scalar.activation(h3t[:, s:s + w], ph[:], Lrelu,
                                     bias=b3t, scale=1.0, alpha=0.01)

            # sweep 3: y = W4 h3 + b4
            for s, w in spans:
                py = ps.tile([O, MACRO], F32, tag="mm", name="py")[:, :w]
                for j in range(s, s + w, MMF):
                    nc.tensor.matmul(py[:, j - s:j - s + MMF], w4tt,
                                     h3t[:, j:j + MMF], start=True, stop=True)
                nc.vector.tensor_scalar_add(yt[:, s:s + w], py[:], b4t)
                nc.sync.dma_start(y[:, s:s + w], yt[:, s:s + w])
    nc.compile()
    return nc


def kernel(fusion_context, seg, W1, b1, Wr1, br1, Wr2, br2, W3, b3, W4, b4):
    global LAST_RESULTS
    fusion_context = np.asarray(fusion_context, dtype=np.float32)
    seg = np.asarray(seg)

    # [B,C,H,W] -> [C, B*H*W]; column n = (b, h, w) row-major
    xcols = np.ascontiguousarray(
        fusion_context.transpose(1, 0, 2, 3).reshape(C, NTOT))
    segf = seg.reshape(-1).astype(np.int64)

    # Route: per class index list, split into two halves -> 8 core shards
    shards = []  # (class_id, column_indices)
    for k in range(K):
        ix = np.nonzero(segf == k)[0]
        h = (len(ix) + 1) // 2
        shards.append((k, ix[:h]))
        shards.append((k, ix[h:]))
    assert len(shards) == NCORES

    cap = max(len(ix) for _, ix in shards)
    cap = max(MMF, -(-cap // MMF) * MMF)  # round up to matmul tile

    if cap not in _nc_cache:
        _nc_cache[cap] = _build(cap)
    nc = _nc_cache[cap]

    f64 = np.float64
    in_maps = []
    for k, ix in shards:
        xs = np.zeros((C, cap), dtype=np.float32)
        xs[:, :len(ix)] = xcols[:, ix]
        V = W1[k].astype(f64).T @ Wr1[k].astype(f64).T    # (Wr1 W1)^T
        T = W1[k].astype(f64).T @ W3[k].astype(f64).T     # (W3 W1)^T
        U = Wr2[k].astype(f64).T @ W3[k].astype(f64).T    # (W3 Wr2)^T
        c = Wr1[k].astype(f64) @ b1[k].astype(f64) + br1[k].astype(f64)
        b3p = W3[k].astype(f64) @ (b1[k].astype(f64) + br2[k].astype(f64)) \
            + b3[k].astype(f64)
        wp = np.concatenate(
            [V, T, U, W4[k].T.astype(f64)], axis=1).astype(np.float32)
        bp = np.zeros((C, 3), dtype=np.float32)
        bp[:, 0] = c
        bp[:, 1] = b3p
        bp[:O, 2] = b4[k]
        in_maps.append({
            "x": xs,
            "wp": np.ascontiguousarray(wp),
            "bp": bp,
        })

    res = run_bass_kernel_spmd(nc, in_maps, core_ids=list(range(NCORES)))
    LAST_RESULTS = res

    out = np.empty((O, NTOT), dtype=np.float32)
    for (k, ix), r in zip(shards, res.results):
        out[:, ix] = r["y"][:, :len(ix)]
    return np.ascontiguousarray(
        out.reshape(O, B, H * W).transpose(1, 0, 2).reshape(B, O, H, W))
